# revision 1
# baseline (speedup 1.0000x reference)
"""HRR binding self-attention kernel for 8 trn2 NeuronCores.

Math: out = irfft(c * rfft(x) * cumsum_s(rfft(x))) @ w_out.T  with c = queries*keyvalues.
Since rfft is linear, cumsum commutes with it: only ONE forward DFT of x is needed;
the causal prefix sum runs in the frequency domain.

Sharding: 8 shards = (batch b in 0..3) x (seq half h in 0..1), 2048 tokens each.
The h=1 shards get the first half's contribution as an initial carry, computed on
host as rfft(x[b, :2048].sum(0)) (O(B*D log D) -- negligible).

Packed real spectrum (2048 rows): rows 0..1024 = Re[0..1024], rows 1025..2047 =
Im[1..1023].  Row 1024 (Nyquist, purely real) rides in the Im-block's first slot
(chunk 8, partition 0); complex multiplies pair chunk i with chunk 8+i on equal
partitions, with a 2-row fixup for the DC/Nyquist slots.

Per-core pipeline (all matmuls bf16 with fp32 PSUM accumulate):
  A) stream 128-token tiles: DFT (x^T tiles @ CS) -> token-major spectrum;
     triangular matmul (spec_tile @ [U|I]) = within-tile cumsum AND transpose to
     freq-major in one shot; carry added as per-partition ACT bias; c-filter as
     per-partition ACT scale; 6 wide DVE ops do the complex multiply; spill qv.
  B) irfft (G @ qv) fused with the output Linear (vals^T @ w_out^T), both
     weight-resident, 512-wide matmuls; fp32 output.
"""

import sys

sys.path.insert(0, "/opt/trn_rl_repo")

import numpy as np
import ml_dtypes

import concourse.bass as bass
import concourse.bacc as bacc
import concourse.mybir as mybir
from concourse.tile import TileContext
from concourse.bass_utils import run_bass_kernel_spmd

BF16 = mybir.dt.bfloat16
F32 = mybir.dt.float32
AF = mybir.ActivationFunctionType

P = 128
D = 2048  # model dims
T = 2048  # tokens per shard
ND = D // P  # 16 d-chunks
NPF = 16  # packed-frequency chunks
NT = T // P  # 16 token tiles
TS = 256  # phase-B token slab
NB = 4  # batch
NS = 4096  # full seq

bf16 = ml_dtypes.bfloat16

_CACHE = {}


def _build_nc(reps: int = 1):
    nc = bacc.Bacc("TRN2", target_bir_lowering=False, debug=False, num_devices=8)
    xT = nc.dram_tensor("xT", [NT, P, ND, P], BF16, kind="ExternalInput")
    CS = nc.dram_tensor("CS", [P, ND, D], BF16, kind="ExternalInput")
    G = nc.dram_tensor("G", [P, NPF, D], BF16, kind="ExternalInput")
    WT = nc.dram_tensor("WT", [P, ND, D], BF16, kind="ExternalInput")
    UI = nc.dram_tensor("UI", [P, 2 * P], BF16, kind="ExternalInput")
    CQ = nc.dram_tensor("CQ", [P, NPF], F32, kind="ExternalInput")
    C0 = nc.dram_tensor("C0", [P, NPF], F32, kind="ExternalInput")
    out = nc.dram_tensor("out", [T, D], F32, kind="ExternalOutput")

    with TileContext(nc) as tc:
        with (
            tc.tile_pool(name="misc", bufs=1) as misc,
            tc.tile_pool(name="dram", bufs=1, space="DRAM") as dramp,
        ):
            ui_sb = misc.tile([P, 2 * P], BF16)
            nc.sync.dma_start(ui_sb[:], UI[:])
            cq_sb = misc.tile([P, NPF], F32)
            nc.sync.dma_start(cq_sb[:], CQ[:])
            c0_sb = misc.tile([P, NPF], F32)
            nc.sync.dma_start(c0_sb[:], C0[:])
            qv_dram = dramp.tile([T // TS, P, NPF, TS], BF16)

            import contextlib

            loop_ctx = (
                tc.For_i(0, reps, 1) if reps > 1 else contextlib.nullcontext()
            )
            with loop_ctx:
                self_body(nc, tc, misc, dramp, ui_sb, cq_sb, c0_sb, qv_dram,
                          CS, G, WT, xT, out)
    nc.finalize()
    return nc


def self_body(nc, tc, misc, dramp, ui_sb, cq_sb, c0_sb, qv_dram, CS, G, WT, xT, out):
    if True:
        if True:
            # ---------------- Phase A ----------------
            with (
                tc.tile_pool(name="constA", bufs=1) as cA,
                tc.tile_pool(name="xt", bufs=3) as xpool,
                tc.tile_pool(name="xh", bufs=3) as xhpool,
                tc.tile_pool(name="sq", bufs=3) as sqpool,
                tc.tile_pool(name="tmp", bufs=2) as tpool,
                tc.tile_pool(name="stage", bufs=3) as stpool,
                tc.tile_pool(name="psA", bufs=4, space="PSUM") as psumA,
                tc.tile_pool(name="psT", bufs=4, space="PSUM") as psumT,
            ):
                cs_sb = cA.tile([P, ND, D], BF16)
                nc.sync.dma_start(cs_sb[:], CS[:])

                carry_prev = None
                stage_tile = None
                for t in range(NT):
                    xt = xpool.tile([P, ND, P], BF16, tag="xt")
                    nc.sync.dma_start(xt[:], xT[t])
                    xh = xhpool.tile([P, D], BF16, tag="xh")
                    psA4 = [psumA.tile([P, 512], F32, tag="psA", name=f"psA{_fq}") for _fq in range(4)]
                    for d in range(ND):
                        for fq in range(4):
                            nc.tensor.matmul(
                                psA4[fq][:],
                                xt[:, d, :],
                                cs_sb[:, d, fq * 512 : (fq + 1) * 512],
                                start=(d == 0),
                                stop=(d == ND - 1),
                            )
                    for fq in range(4):
                        if fq % 2 == 0:
                            nc.vector.tensor_copy(
                                xh[:, fq * 512 : (fq + 1) * 512], psA4[fq][:]
                            )
                        else:
                            nc.scalar.copy(
                                xh[:, fq * 512 : (fq + 1) * 512], psA4[fq][:]
                            )

                    S_sb = sqpool.tile([P, NPF, P], BF16, tag="S")
                    Q_sb = sqpool.tile([P, NPF, P], BF16, tag="Q")
                    for pf in range(NPF):
                        pst = psumT.tile([P, 2 * P], F32, tag="psT")
                        nc.tensor.matmul(
                            pst[:],
                            xh[:, pf * P : (pf + 1) * P],
                            ui_sb[:],
                            start=True,
                            stop=True,
                        )
                        carry_ap = (
                            c0_sb[:, pf : pf + 1]
                            if t == 0
                            else carry_prev[:, pf, P - 1 : P]
                        )
                        nc.scalar.activation(
                            S_sb[:, pf, :], pst[:, 0:P], AF.Identity, bias=carry_ap
                        )
                        nc.scalar.activation(
                            Q_sb[:, pf, :],
                            pst[:, P : 2 * P],
                            AF.Copy,
                            scale=cq_sb[:, pf : pf + 1],
                        )
                    carry_prev = S_sb

                    if t % 2 == 0:
                        stage_tile = stpool.tile([P, NPF, TS], BF16, tag="stage")
                    qv = stage_tile[:, :, (t % 2) * P : (t % 2 + 1) * P]
                    t1 = tpool.tile([P, 8, P], F32, tag="t1")
                    t2 = tpool.tile([P, 8, P], F32, tag="t2")
                    nc.vector.tensor_mul(t1[:], Q_sb[:, 0:8, :], S_sb[:, 0:8, :])
                    nc.vector.tensor_mul(t2[:], Q_sb[:, 8:16, :], S_sb[:, 8:16, :])
                    nc.vector.tensor_sub(qv[:, 0:8, :], t1[:], t2[:])
                    t3 = tpool.tile([P, 8, P], F32, tag="t1")
                    t4 = tpool.tile([P, 8, P], F32, tag="t2")
                    nc.vector.tensor_mul(t3[:], Q_sb[:, 0:8, :], S_sb[:, 8:16, :])
                    nc.vector.tensor_mul(t4[:], Q_sb[:, 8:16, :], S_sb[:, 0:8, :])
                    nc.vector.tensor_add(qv[:, 8:16, :], t3[:], t4[:])
                    # DC (chunk 0 row 0) and Nyquist (chunk 8 row 0) are purely real
                    nc.vector.tensor_mul(qv[0:1, 0, :], Q_sb[0:1, 0, :], S_sb[0:1, 0, :])
                    nc.vector.tensor_mul(qv[0:1, 8, :], Q_sb[0:1, 8, :], S_sb[0:1, 8, :])

                    if t % 2 == 1:
                        nc.sync.dma_start(qv_dram[t // 2], stage_tile[:])

            # ---------------- Phase B ----------------
            with (
                tc.tile_pool(name="constB", bufs=1) as cB,
                tc.tile_pool(name="qs", bufs=2) as qspool,
                tc.tile_pool(name="vals", bufs=1) as vpool,
                tc.tile_pool(name="osb", bufs=2) as opool,
                tc.tile_pool(name="psB", bufs=2, space="PSUM") as psumB,
                tc.tile_pool(name="psC", bufs=4, space="PSUM") as psumC,
            ):
                g_sb = cB.tile([P, NPF, D], BF16)
                nc.sync.dma_start(g_sb[:], G[:])
                wt_sb = cB.tile([P, ND, D], BF16)
                nc.sync.dma_start(wt_sb[:], WT[:])

                TSB = 512
                for s in range(T // TSB):
                    qs = qspool.tile([P, NPF, TSB], BF16, tag="qs")
                    nc.sync.dma_start(qs[:, :, 0:TS], qv_dram[2 * s])
                    nc.sync.dma_start(qs[:, :, TS : 2 * TS], qv_dram[2 * s + 1])
                    vals = vpool.tile([P, ND, TSB], BF16, tag="vals")
                    for dt in range(ND):
                        psb = psumB.tile([P, TSB], F32, tag="psB")
                        for pf in range(NPF):
                            nc.tensor.matmul(
                                psb[:],
                                g_sb[:, pf, dt * P : (dt + 1) * P],
                                qs[:, pf, :],
                                start=(pf == 0),
                                stop=(pf == NPF - 1),
                            )
                        nc.vector.tensor_copy(vals[:, dt, :], psb[:])
                    for ts_ in range(TSB // P):
                        osb = opool.tile([P, D], F32, tag="osb")
                        for e in range(4):
                            psc = psumC.tile([P, 512], F32, tag="psC")
                            for dt in range(ND):
                                nc.tensor.matmul(
                                    psc[:],
                                    vals[:, dt, ts_ * P : (ts_ + 1) * P],
                                    wt_sb[:, dt, e * 512 : (e + 1) * 512],
                                    start=(dt == 0),
                                    stop=(dt == ND - 1),
                                )
                            if e % 2 == 0:
                                nc.scalar.copy(
                                    osb[:, e * 512 : (e + 1) * 512], psc[:]
                                )
                            else:
                                nc.vector.tensor_copy(
                                    osb[:, e * 512 : (e + 1) * 512], psc[:]
                                )
                        r0 = s * TSB + ts_ * P
                        nc.sync.dma_start(out[r0 : r0 + P, :], osb[:])


def _chunked(m):
    """[rows, cols] -> [P, rows//P, cols] with row r at [r % P, r // P]."""
    r, c = m.shape
    return np.ascontiguousarray(m.reshape(r // P, P, c).transpose(1, 0, 2))


def _pack_spec(re, im):
    """re[1025], im[1025] -> packed [2048]: re[0..1024] then im[1..1023]."""
    return np.concatenate([re, im[1:1024]])


def _constants():
    if "consts" in _CACHE:
        return _CACHE["consts"]
    d = np.arange(D, dtype=np.float64)
    f = np.arange(D // 2 + 1, dtype=np.float64)
    ang = 2.0 * np.pi / D * np.outer(d, f)  # [D, 1025]
    cos, sin = np.cos(ang), np.sin(ang)
    CSf = np.concatenate([cos, -sin[:, 1:1024]], axis=1)  # [D, D]
    alpha = np.full(1025, 2.0)
    alpha[0] = alpha[1024] = 1.0
    Gf = np.concatenate(
        [(alpha[:, None] * cos.T) / D, (-2.0 * sin[:, 1:1024].T) / D], axis=0
    )  # [D packed, D]
    U = np.triu(np.ones((P, P)))
    UI = np.concatenate([U, np.eye(P)], axis=1)
    consts = {
        "CS": _chunked(CSf.astype(np.float32)).astype(bf16),
        "G": _chunked(Gf.astype(np.float32)).astype(bf16),
        "UI": UI.astype(bf16),
    }
    _CACHE["consts"] = consts
    return consts


def kernel(x, queries, keyvalues, w_out):
    x = np.asarray(x, dtype=np.float32)
    queries = np.asarray(queries, dtype=np.float32)
    keyvalues = np.asarray(keyvalues, dtype=np.float32)
    w_out = np.asarray(w_out, dtype=np.float32)

    if "nc" not in _CACHE:
        _CACHE["nc"] = _build_nc()
    nc = _CACHE["nc"]
    consts = _constants()

    c = (queries * keyvalues).reshape(-1)  # [1025]
    cq = _chunked(_pack_spec(c, c).astype(np.float32)[:, None])[:, :, 0]  # [P, NPF]
    WTc = _chunked(np.ascontiguousarray(w_out.T).astype(np.float32)).astype(bf16)

    in_maps = []
    shards = []
    for b in range(NB):
        for h in range(2):
            shards.append((b, h))
            xs = x[b, h * T : (h + 1) * T]  # [T, D]
            xT3 = _chunked(np.ascontiguousarray(xs.T))  # [P, ND, T]
            xTc = np.ascontiguousarray(
                xT3.reshape(P, ND, NT, P).transpose(2, 0, 1, 3)
            ).astype(bf16)
            if h == 0:
                c0 = np.zeros((P, NPF), np.float32)
            else:
                F = np.fft.rfft(x[b, :T].sum(axis=0).astype(np.float64))
                c0 = _chunked(
                    _pack_spec(F.real, F.imag).astype(np.float32)[:, None]
                )[:, :, 0]
            in_maps.append(
                {
                    "xT": xTc,
                    "CS": consts["CS"],
                    "G": consts["G"],
                    "WT": WTc,
                    "UI": consts["UI"],
                    "CQ": np.ascontiguousarray(cq),
                    "C0": np.ascontiguousarray(c0),
                }
            )

    global _LAST_IN_MAPS
    _LAST_IN_MAPS = in_maps
    res = run_bass_kernel_spmd(nc, in_maps, core_ids=list(range(8)))
    y = np.empty((NB, NS, D), np.float32)
    for i, (b, h) in enumerate(shards):
        y[b, h * T : (h + 1) * T] = res.results[i]["out"]
    return y



# revision 10
# speedup vs baseline: 85.4734x; 85.4734x over previous
"""HRR binding self-attention kernel for 8 trn2 NeuronCores.

Math: out = irfft(c * rfft(x) * cumsum_s(rfft(x))) @ w_out.T  with c = queries*keyvalues.
rfft is linear so cumsum commutes with it: one forward DFT, prefix sum in the
frequency domain.

Sharding: 8 shards = (batch b in 0..3) x (seq half h in 0..1), 2048 tokens each.
h=1 shards get the first half's contribution as an initial carry, computed on
host as rfft(x[b, :2048].sum(0)).

Two algorithmic wins over the v1 kernel:
  1. Radix-2 split of the forward DFT: xhat = E + TO where E/TO are the
     partial DFT sums over even/odd features.  Each is Hermitian-symmetric, so
     only 1024 matmul columns per half are computed (vs 2048 for the full
     spectrum); the missing half-spectrum is reconstructed by a 4-slice DVE
     combine (pure adds/subs -- the conjugate reuse is absorbed into a
     permuted packing F of the spectrum rows, which downstream constants
     (CQ/C0/GW) are built against).  DFT matmul cost halves.
  2. irfft and the output Linear fused into ONE matmul: out = qv @ GW with
     GW = G_packed @ w_out.T precomputed on host.  Removes the irfft matmul
     stage and the qv DRAM round-trip entirely.

Packed spectrum (2048 rows, F-map): rows 0..1023 = Re[fm[p]], row 1024 =
Re[Nyquist], rows 1025..2047 = Im[fm[p-1024]], where fm = [0..512, 1023..513].
Complex multiplies pair chunk i with chunk 8+i on equal partitions; rows 0 and
1024 (DC/Nyquist, purely real) get a 2-row fixup.

Single merged per-tile pipeline (software-pipelined, all matmuls bf16 with
fp32 PSUM): DFT(t) -> combine(t) [DVE] -> tri(t-1) [PE] -> S/Q(t-1) [ACT] ->
cmult(t-1) [DVE] -> GW(t-2) [PE] -> out copy + DMA.  qv lives only in SBUF.
"""

import sys

sys.path.insert(0, "/opt/trn_rl_repo")

import numpy as np
import ml_dtypes

import concourse.bass as bass
import concourse.bacc as bacc
import concourse.mybir as mybir
from concourse.tile import TileContext
from concourse.bass_utils import run_bass_kernel_spmd

BF16 = mybir.dt.bfloat16
F32 = mybir.dt.float32
AF = mybir.ActivationFunctionType

P = 128
D = 2048  # model dims
T = 2048  # tokens per shard
NPF = 16  # packed-frequency chunks
NT = T // P  # 16 token tiles
NB = 4  # batch
NS = 4096  # full seq

bf16 = ml_dtypes.bfloat16

_CACHE = {}


def _build_nc(reps: int = 1):
    nc = bacc.Bacc("TRN2", target_bir_lowering=False, debug=False, num_devices=8)
    xT = nc.dram_tensor("xT", [NT, P, NPF, P], BF16, kind="ExternalInput")
    CSE = nc.dram_tensor("CSE", [P, 8, 1024], BF16, kind="ExternalInput")
    CSO = nc.dram_tensor("CSO", [P, 8, 1024], BF16, kind="ExternalInput")
    GW = nc.dram_tensor("GW", [P, NPF, D], BF16, kind="ExternalInput")
    UI = nc.dram_tensor("UI", [P, 2 * P], BF16, kind="ExternalInput")
    CQ = nc.dram_tensor("CQ", [P, NPF], F32, kind="ExternalInput")
    C0 = nc.dram_tensor("C0", [P, NPF], F32, kind="ExternalInput")
    out = nc.dram_tensor("out", [T, D], F32, kind="ExternalOutput")

    with TileContext(nc) as tc:
        with tc.tile_pool(name="misc", bufs=1) as misc:
            ui_sb = misc.tile([P, 2 * P], BF16)
            nc.sync.dma_start(ui_sb[:], UI[:])
            cq_sb = misc.tile([P, NPF], F32)
            nc.sync.dma_start(cq_sb[:], CQ[:])
            c0_sb = misc.tile([P, NPF], F32)
            nc.sync.dma_start(c0_sb[:], C0[:])

            import contextlib

            loop_ctx = tc.For_i(0, reps, 1) if reps > 1 else contextlib.nullcontext()
            with loop_ctx:
                self_body(nc, tc, ui_sb, cq_sb, c0_sb, CSE, CSO, GW, xT, out)
    nc.finalize()
    return nc


def self_body(nc, tc, ui_sb, cq_sb, c0_sb, CSE, CSO, GW, xT, out):
    with (
        tc.tile_pool(name="const", bufs=1) as cpool,
        tc.tile_pool(name="xt", bufs=3) as xpool,
        tc.tile_pool(name="xh", bufs=2) as xhpool,
        tc.tile_pool(name="sbe", bufs=2) as sbepool,
        tc.tile_pool(name="sq", bufs=3) as sqpool,
        tc.tile_pool(name="tmp", bufs=2) as tpool,
        tc.tile_pool(name="qv", bufs=3) as qvpool,
        tc.tile_pool(name="osb", bufs=2) as opool,
        tc.tile_pool(name="psD", bufs=2, space="PSUM") as psumD,
        tc.tile_pool(name="psT", bufs=4, space="PSUM") as psumT,
        tc.tile_pool(name="psG", bufs=2, space="PSUM") as psumG,
    ):
        cse_sb = cpool.tile([P, 8, 1024], BF16)
        nc.sync.dma_start(cse_sb[:], CSE[:])
        cso_sb = cpool.tile([P, 8, 1024], BF16)
        nc.sync.dma_start(cso_sb[:], CSO[:])
        gw_sb = cpool.tile([P, NPF, D], BF16)
        nc.sync.dma_start(gw_sb[:], GW[:])

        xh_hist = {}
        S_hist = {}
        Q_hist = {}
        qv_hist = {}

        LAG_TRI = 1  # tri stage runs 1 tile behind DFT
        LAG_GW = 2  # GW stage runs 2 tiles behind DFT

        for it in range(NT + LAG_GW):
            # ---------- stage 1a: DFT(t) half 0 + combine ----------
            t = it
            if t < NT:
                xt = xpool.tile([P, NPF, P], BF16, tag="xt")
                nc.sync.dma_start(xt[:], xT[t])
                xh = xhpool.tile([P, D], BF16, tag="xh")
                psE0 = psumD.tile([P, 512], F32, tag="psD", name="psE0")
                psO0 = psumD.tile([P, 512], F32, tag="psD", name="psO0")
                for c in range(8):
                    st, sp = c == 0, c == 7
                    nc.tensor.matmul(
                        psE0[:], xt[:, c, :], cse_sb[:, c, 0:512], start=st, stop=sp
                    )
                    nc.tensor.matmul(
                        psO0[:], xt[:, 8 + c, :], cso_sb[:, c, 0:512], start=st, stop=sp
                    )
                sbE0 = sbepool.tile([P, 512], F32, tag="sbE")
                nc.scalar.copy(sbE0[:], psE0[:])
                nc.vector.tensor_add(xh[:, 0:512], sbE0[:, 0:512], psO0[:, 0:512])
                nc.vector.tensor_sub(xh[:, 513:1024], sbE0[:, 1:512], psO0[:, 1:512])
                nc.vector.tensor_sub(xh[:, 1024:1025], sbE0[:, 0:1], psO0[:, 0:1])
                xh_hist[t] = xh

            # ---------- stage 2: tri + S/Q + cmult (t - LAG_TRI) ----------
            u = it - LAG_TRI
            if 0 <= u < NT:
                xh_u = xh_hist.pop(u)
                S_sb = sqpool.tile([P, NPF, P], BF16, tag="S")
                Q_sb = sqpool.tile([P, NPF, P], BF16, tag="Q")
                for pf in range(NPF):
                    pst = psumT.tile([P, 2 * P], F32, tag="psT")
                    nc.tensor.matmul(
                        pst[:],
                        xh_u[:, pf * P : (pf + 1) * P],
                        ui_sb[:],
                        start=True,
                        stop=True,
                    )
                    carry_ap = (
                        c0_sb[:, pf : pf + 1]
                        if u == 0
                        else S_hist[u - 1][:, pf, P - 1 : P]
                    )
                    nc.scalar.activation(
                        S_sb[:, pf, :], pst[:, 0:P], AF.Identity, bias=carry_ap
                    )
                    nc.scalar.activation(
                        Q_sb[:, pf, :],
                        pst[:, P : 2 * P],
                        AF.Copy,
                        scale=cq_sb[:, pf : pf + 1],
                    )
                S_hist.pop(u - 1, None)
                S_hist[u] = S_sb
                Q_hist[u] = Q_sb

                qv = qvpool.tile([P, NPF, P], BF16, tag="qv")
                t1 = tpool.tile([P, 8, P], F32, tag="t1")
                t2 = tpool.tile([P, 8, P], F32, tag="t2")
                nc.vector.tensor_mul(t1[:], Q_sb[:, 0:8, :], S_sb[:, 0:8, :])
                nc.vector.tensor_mul(t2[:], Q_sb[:, 8:16, :], S_sb[:, 8:16, :])
                nc.vector.tensor_sub(qv[:, 0:8, :], t1[:], t2[:])
                t3 = tpool.tile([P, 8, P], F32, tag="t1")
                t4 = tpool.tile([P, 8, P], F32, tag="t2")
                nc.vector.tensor_mul(t3[:], Q_sb[:, 0:8, :], S_sb[:, 8:16, :])
                nc.vector.tensor_mul(t4[:], Q_sb[:, 8:16, :], S_sb[:, 0:8, :])
                nc.vector.tensor_add(qv[:, 8:16, :], t3[:], t4[:])
                # DC (chunk 0 row 0) and Nyquist (chunk 8 row 0) purely real
                nc.vector.tensor_mul(qv[0:1, 0, :], Q_sb[0:1, 0, :], S_sb[0:1, 0, :])
                nc.vector.tensor_mul(qv[0:1, 8, :], Q_sb[0:1, 8, :], S_sb[0:1, 8, :])
                Q_hist.pop(u, None)
                qv_hist[u] = qv

            # ---------- stage 1b: DFT(t) half 1 + combine ----------
            if t < NT:
                psE1 = psumD.tile([P, 512], F32, tag="psD", name="psE1")
                psO1 = psumD.tile([P, 512], F32, tag="psD", name="psO1")
                for c in range(8):
                    st, sp = c == 0, c == 7
                    nc.tensor.matmul(
                        psE1[:], xt[:, c, :], cse_sb[:, c, 512:1024], start=st, stop=sp
                    )
                    nc.tensor.matmul(
                        psO1[:],
                        xt[:, 8 + c, :],
                        cso_sb[:, c, 512:1024],
                        start=st,
                        stop=sp,
                    )
                sbE1 = sbepool.tile([P, 512], F32, tag="sbE")
                nc.scalar.copy(sbE1[:], psE1[:])
                nc.vector.tensor_add(xh[:, 1025:1536], sbE1[:, 1:512], psO1[:, 1:512])
                nc.vector.tensor_sub(xh[:, 1537:2048], psO1[:, 1:512], sbE1[:, 1:512])
                nc.scalar.copy(xh[:, 512:513], sbE1[:, 0:1])
                nc.scalar.copy(xh[:, 1536:1537], psO1[:, 0:1])

            # ---------- stage 3: fused GW matmul (t - LAG_GW) ----------
            v = it - LAG_GW
            if v >= 0:
                qv = qv_hist.pop(v)
                osb = opool.tile([P, D], F32, tag="osb")
                for e in range(4):
                    psg = psumG.tile([P, 512], F32, tag="psG")
                    for pf in range(NPF):
                        nc.tensor.matmul(
                            psg[:],
                            qv[:, pf, :],
                            gw_sb[:, pf, e * 512 : (e + 1) * 512],
                            start=(pf == 0),
                            stop=(pf == NPF - 1),
                        )
                    if e % 2 == 0:
                        nc.scalar.copy(osb[:, e * 512 : (e + 1) * 512], psg[:])
                    else:
                        nc.vector.tensor_copy(osb[:, e * 512 : (e + 1) * 512], psg[:])
                nc.sync.dma_start(out[v * P : (v + 1) * P, :], osb[:])


def _chunked(m):
    """[rows, cols] -> [P, rows//P, cols] with row r at [r % P, r // P]."""
    r, c = m.shape
    return np.ascontiguousarray(m.reshape(r // P, P, c).transpose(1, 0, 2))


_FM = np.concatenate([np.arange(513), np.arange(1023, 512, -1)])  # len 1024


def _pack_F(re, im):
    """re[1025], im[1025] -> packed [2048] under the F-map layout."""
    v = np.empty(2048)
    v[0:1024] = re[_FM]
    v[1024] = re[1024]
    v[1025:2048] = im[_FM[1:1024]]
    return v


def _consts():
    if "consts" in _CACHE:
        return _CACHE["consts"]

    def partial_mat(dd):
        m = np.empty((len(dd), 1024))
        m[:, 0:513] = np.cos(2 * np.pi * np.outer(dd, np.arange(513)) / D)
        m[:, 513:1024] = -np.sin(2 * np.pi * np.outer(dd, np.arange(1, 512)) / D)
        return m

    CSE = partial_mat(np.arange(0, D, 2))
    CSO = partial_mat(np.arange(1, D, 2))
    CSO[:, 512] = -np.sin(2 * np.pi * np.arange(1, D, 2) * 512 / D)

    U = np.triu(np.ones((P, P)))
    UI = np.concatenate([U, np.eye(P)], axis=1)
    consts = {
        "CSE": _chunked(CSE.astype(np.float32)).astype(bf16),
        "CSO": _chunked(CSO.astype(np.float32)).astype(bf16),
        "UI": UI.astype(bf16),
    }
    _CACHE["consts"] = consts
    return consts


def _gw_matrix(w_out):
    """GW = G_F @ w_out.T, chunked [P, NPF, D] bf16."""
    f_of_row = np.empty(2048, dtype=np.int64)
    f_of_row[0:1024] = _FM
    f_of_row[1024] = 1024
    f_of_row[1025:2048] = _FM[1:1024]
    alpha = np.where((f_of_row == 0) | (f_of_row == 1024), 1.0, 2.0)
    ang = 2 * np.pi / D * np.outer(f_of_row, np.arange(D))
    G_F = np.empty((2048, D), np.float64)
    G_F[0:1025] = alpha[0:1025, None] * np.cos(ang[0:1025]) / D
    G_F[1025:] = -2.0 * np.sin(ang[1025:]) / D
    GWm = (G_F.astype(np.float32) @ w_out.T.astype(np.float32)).astype(np.float32)
    return _chunked(GWm).astype(bf16)


def kernel(x, queries, keyvalues, w_out):
    x = np.asarray(x, dtype=np.float32)
    queries = np.asarray(queries, dtype=np.float32)
    keyvalues = np.asarray(keyvalues, dtype=np.float32)
    w_out = np.asarray(w_out, dtype=np.float32)

    if "nc" not in _CACHE:
        _CACHE["nc"] = _build_nc()
    nc = _CACHE["nc"]
    consts = _consts()

    c = (queries * keyvalues).reshape(-1)  # [1025]
    cq = _chunked(_pack_F(c, c).astype(np.float32)[:, None])[:, :, 0]  # [P, NPF]
    GWc = _gw_matrix(w_out)

    in_maps = []
    shards = []
    for b in range(NB):
        for h in range(2):
            shards.append((b, h))
            xs = x[b, h * T : (h + 1) * T]  # [T, D]
            xsT = np.ascontiguousarray(xs.T)  # [D, T]
            xe = _chunked(np.ascontiguousarray(xsT[0::2]))  # [P, 8, T]
            xo = _chunked(np.ascontiguousarray(xsT[1::2]))  # [P, 8, T]
            xfull = np.concatenate([xe, xo], axis=1)  # [P, 16, T]
            xTc = np.ascontiguousarray(
                xfull.reshape(P, NPF, NT, P).transpose(2, 0, 1, 3)
            ).astype(bf16)
            if h == 0:
                c0 = np.zeros((P, NPF), np.float32)
            else:
                F = np.fft.rfft(x[b, :T].sum(axis=0).astype(np.float64))
                c0 = _chunked(
                    _pack_F(F.real, F.imag).astype(np.float32)[:, None]
                )[:, :, 0]
            in_maps.append(
                {
                    "xT": xTc,
                    "CSE": consts["CSE"],
                    "CSO": consts["CSO"],
                    "GW": GWc,
                    "UI": consts["UI"],
                    "CQ": np.ascontiguousarray(cq),
                    "C0": np.ascontiguousarray(c0),
                }
            )

    global _LAST_IN_MAPS
    _LAST_IN_MAPS = in_maps
    res = run_bass_kernel_spmd(nc, in_maps, core_ids=list(range(8)))
    y = np.empty((NB, NS, D), np.float32)
    for i, (b, h) in enumerate(shards):
        y[b, h * T : (h + 1) * T] = res.results[i]["out"]
    return y


# revision 12
# speedup vs baseline: 100.8605x; 1.1800x over previous
"""HRR binding self-attention kernel for 8 trn2 NeuronCores — radix-4 DFT.

Same structure as the radix-2 version, but the forward DFT is factored one
level further: x is split into 4 stride-4 subsequences whose partial DFTs
B0..B3 (512 matmul columns each, Hermitian-unique) are combined in two
slice-add levels (B0,B2 -> E block; B1,B3 -> TO block; E,TO -> packed
spectrum).  All conjugate reuse is absorbed into a permuted packing map fm,
which the host-built constants (CQ/C0/GW) are generated against.
DFT matmul cost: 8192 cy/tile (vs 16384 radix-2, 32768 direct).
"""

import sys

sys.path.insert(0, "/opt/trn_rl_repo")

import numpy as np
import ml_dtypes

import concourse.bass as bass
import concourse.bacc as bacc
import concourse.mybir as mybir
from concourse.tile import TileContext
from concourse.bass_utils import run_bass_kernel_spmd

BF16 = mybir.dt.bfloat16
F32 = mybir.dt.float32
AF = mybir.ActivationFunctionType

P = 128
D = 2048
T = 2048
NPF = 16
NT = T // P
NB = 4
NS = 4096

bf16 = ml_dtypes.bfloat16

_CACHE = {}


def _build_nc(reps: int = 1):
    nc = bacc.Bacc("TRN2", target_bir_lowering=False, debug=False, num_devices=8)
    xT = nc.dram_tensor("xT", [NT, P, NPF, P], BF16, kind="ExternalInput")
    CB = nc.dram_tensor("CB", [P, NPF, 512], BF16, kind="ExternalInput")
    GW = nc.dram_tensor("GW", [4, P, NPF, 512], BF16, kind="ExternalInput")
    UI = nc.dram_tensor("UI", [P, 2 * P], BF16, kind="ExternalInput")
    CQ = nc.dram_tensor("CQ", [P, NPF], F32, kind="ExternalInput")
    C0 = nc.dram_tensor("C0", [P, NPF], F32, kind="ExternalInput")
    out = nc.dram_tensor("out", [T, D], F32, kind="ExternalOutput")

    with TileContext(nc) as tc:
        with tc.tile_pool(name="misc", bufs=1) as misc:
            ui_sb = misc.tile([P, 2 * P], BF16)
            nc.sync.dma_start(ui_sb[:], UI[:])
            cq_sb = misc.tile([P, NPF], F32)
            nc.sync.dma_start(cq_sb[:], CQ[:])
            c0_sb = misc.tile([P, NPF], F32)
            nc.sync.dma_start(c0_sb[:], C0[:])

            import contextlib

            loop_ctx = tc.For_i(0, reps, 1) if reps > 1 else contextlib.nullcontext()
            with loop_ctx:
                self_body(nc, tc, ui_sb, cq_sb, c0_sb, CB, GW, xT, out)
    nc.finalize()
    return nc


def self_body(nc, tc, ui_sb, cq_sb, c0_sb, CB, GW, xT, out):
    with (
        tc.tile_pool(name="const", bufs=1) as cpool,
        tc.tile_pool(name="xt", bufs=4) as xpool,
        tc.tile_pool(name="xh", bufs=2) as xhpool,
        tc.tile_pool(name="eto", bufs=2) as etopool,
        tc.tile_pool(name="sbb", bufs=2) as sbbpool,
        tc.tile_pool(name="sq", bufs=3) as sqpool,
        tc.tile_pool(name="tmp", bufs=2) as tpool,
        tc.tile_pool(name="qv", bufs=6) as qvpool,
        tc.tile_pool(name="osb", bufs=2) as opool,
        tc.tile_pool(name="psD", bufs=2, space="PSUM") as psumD,
        tc.tile_pool(name="psT", bufs=4, space="PSUM") as psumT,
        tc.tile_pool(name="psGa", bufs=1, space="PSUM") as psumGa,
        tc.tile_pool(name="psGb", bufs=1, space="PSUM") as psumGb,
    ):
        # sync-queue order tuned so stage 1a(0) (needs xt0 + cb j=0,2) can
        # start as early as possible
        xt_hist = {}
        cb_sb = cpool.tile([P, NPF, 512], BF16)
        xt_pre = xpool.tile([P, NPF, P], BF16, tag="xt", name="xtpre0")
        nc.sync.dma_start(xt_pre[:], xT[0])
        xt_hist[0] = xt_pre
        for j in (0, 2):
            nc.sync.dma_start(
                cb_sb[:, 4 * j : 4 * j + 4, :], CB[:, 4 * j : 4 * j + 4, :]
            )
        xt_pre1 = xpool.tile([P, NPF, P], BF16, tag="xt", name="xtpre1")
        nc.sync.dma_start(xt_pre1[:], xT[1])
        xt_hist[1] = xt_pre1
        for j in (1, 3):
            nc.sync.dma_start(
                cb_sb[:, 4 * j : 4 * j + 4, :], CB[:, 4 * j : 4 * j + 4, :]
            )
        # gw in column quarters, each its own tile (own dep tracking);
        # the DMAs are emitted one per iteration (it=0..3) inside the loop
        gw_q = [
            cpool.tile([P, NPF, 512], BF16, name=f"gwq{q}") for q in range(4)
        ]

        xh_hist = {}
        eto_hist = {}
        S_hist = {}
        Q_hist = {}
        qv_hist = {}

        LAG_TRI = 1
        LAG_GW = 4

        for it in range(NT + LAG_GW):
            # ---------- stage 1a: B0/B2 matmuls + E-block combine ----------
            t = it
            if t < NT:
                if t + 2 < NT:
                    xt_n = xpool.tile([P, NPF, P], BF16, tag="xt")
                    nc.sync.dma_start(xt_n[:], xT[t + 2])
                    xt_hist[t + 2] = xt_n
                xt = xt_hist.pop(t)
                xh = xhpool.tile([P, D], BF16, tag="xh")
                E_sb = etopool.tile([P, 1024], BF16, tag="E")
                psB0 = psumD.tile([P, 512], F32, tag="psD", name="psB0")
                psB2 = psumD.tile([P, 512], F32, tag="psD", name="psB2")
                for c in range(4):
                    st, sp = c == 0, c == 3
                    nc.tensor.matmul(
                        psB0[:], xt[:, c, :], cb_sb[:, c, :], start=st, stop=sp
                    )
                    nc.tensor.matmul(
                        psB2[:], xt[:, 8 + c, :], cb_sb[:, 8 + c, :], start=st, stop=sp
                    )
                sbB0 = sbbpool.tile([P, 512], F32, tag="sbB")
                nc.scalar.copy(sbB0[:], psB0[:])
                nc.vector.tensor_add(E_sb[:, 0:256], sbB0[:, 0:256], psB2[:, 0:256])
                nc.scalar.copy(E_sb[:, 256:257], sbB0[:, 256:257])
                nc.vector.tensor_sub(E_sb[:, 257:512], sbB0[:, 1:256], psB2[:, 1:256])
                nc.vector.tensor_sub(E_sb[:, 512:513], sbB0[:, 0:1], psB2[:, 0:1])
                nc.vector.tensor_add(
                    E_sb[:, 513:768], sbB0[:, 257:512], psB2[:, 256:511]
                )
                nc.scalar.copy(E_sb[:, 768:769], psB2[:, 511:512])
                nc.vector.tensor_sub(
                    E_sb[:, 769:1024], psB2[:, 256:511], sbB0[:, 257:512]
                )
                eto_hist[t] = E_sb
                xh_hist[t] = xh

            # ---------- stage 2: tri + S/Q + cmult (t - LAG_TRI) ----------
            u = it - LAG_TRI
            if 0 <= u < NT:
                xh_u = xh_hist.pop(u)
                S_sb = sqpool.tile([P, NPF, P], BF16, tag="S")
                Q_sb = sqpool.tile([P, NPF, P], BF16, tag="Q")
                for pf in range(NPF):
                    pst = psumT.tile([P, 2 * P], F32, tag="psT")
                    nc.tensor.matmul(
                        pst[:],
                        xh_u[:, pf * P : (pf + 1) * P],
                        ui_sb[:],
                        start=True,
                        stop=True,
                    )
                    carry_ap = (
                        c0_sb[:, pf : pf + 1]
                        if u == 0
                        else S_hist[u - 1][:, pf, P - 1 : P]
                    )
                    nc.scalar.activation(
                        S_sb[:, pf, :], pst[:, 0:P], AF.Identity, bias=carry_ap
                    )
                    nc.scalar.activation(
                        Q_sb[:, pf, :],
                        pst[:, P : 2 * P],
                        AF.Copy,
                        scale=cq_sb[:, pf : pf + 1],
                    )
                S_hist.pop(u - 1, None)
                S_hist[u] = S_sb
                Q_hist[u] = Q_sb

                qv = qvpool.tile([P, NPF, P], BF16, tag="qv")
                t1 = tpool.tile([P, 8, P], BF16, tag="t1")
                t2 = tpool.tile([P, 8, P], BF16, tag="t2")
                nc.vector.tensor_mul(t1[:], Q_sb[:, 0:8, :], S_sb[:, 0:8, :])
                nc.vector.tensor_mul(t2[:], Q_sb[:, 8:16, :], S_sb[:, 8:16, :])
                nc.vector.tensor_sub(qv[:, 0:8, :], t1[:], t2[:])
                t3 = tpool.tile([P, 8, P], BF16, tag="t1")
                t4 = tpool.tile([P, 8, P], BF16, tag="t2")
                nc.vector.tensor_mul(t3[:], Q_sb[:, 0:8, :], S_sb[:, 8:16, :])
                nc.vector.tensor_mul(t4[:], Q_sb[:, 8:16, :], S_sb[:, 0:8, :])
                nc.vector.tensor_add(qv[:, 8:16, :], t3[:], t4[:])
                nc.vector.tensor_mul(qv[0:1, 0, :], Q_sb[0:1, 0, :], S_sb[0:1, 0, :])
                nc.vector.tensor_mul(qv[0:1, 8, :], Q_sb[0:1, 8, :], S_sb[0:1, 8, :])
                Q_hist.pop(u, None)
                qv_hist[u] = qv

            # ---------- stage 1b: B1/B3 + TO combine + level-2 ----------
            if t < NT:
                E_sb = eto_hist.pop(t)
                TO_sb = etopool.tile([P, 1024], BF16, tag="TO")
                psB1 = psumD.tile([P, 512], F32, tag="psD", name="psB1")
                psB3 = psumD.tile([P, 512], F32, tag="psD", name="psB3")
                for c in range(4):
                    st, sp = c == 0, c == 3
                    nc.tensor.matmul(
                        psB1[:], xt[:, 4 + c, :], cb_sb[:, 4 + c, :], start=st, stop=sp
                    )
                    nc.tensor.matmul(
                        psB3[:],
                        xt[:, 12 + c, :],
                        cb_sb[:, 12 + c, :],
                        start=st,
                        stop=sp,
                    )
                sbB1 = sbbpool.tile([P, 512], F32, tag="sbB")
                nc.scalar.copy(sbB1[:], psB1[:])
                nc.vector.tensor_add(TO_sb[:, 0:256], sbB1[:, 0:256], psB3[:, 0:256])
                nc.vector.tensor_sub(
                    TO_sb[:, 256:257], psB3[:, 511:512], sbB1[:, 511:512]
                )
                nc.vector.tensor_sub(
                    TO_sb[:, 257:512], psB3[:, 256:511], sbB1[:, 256:511]
                )
                nc.vector.tensor_sub(TO_sb[:, 512:513], psB3[:, 0:1], sbB1[:, 0:1])
                nc.vector.tensor_add(
                    TO_sb[:, 513:768], sbB1[:, 256:511], psB3[:, 256:511]
                )
                nc.vector.tensor_add(
                    TO_sb[:, 768:769], sbB1[:, 511:512], psB3[:, 511:512]
                )
                nc.vector.tensor_sub(TO_sb[:, 769:1024], psB3[:, 1:256], sbB1[:, 1:256])
                # level-2 combine (both operands SBUF bf16)
                nc.vector.tensor_add(xh[:, 0:512], E_sb[:, 0:512], TO_sb[:, 0:512])
                nc.scalar.copy(xh[:, 512:513], E_sb[:, 512:513])
                nc.vector.tensor_sub(xh[:, 513:1024], E_sb[:, 1:512], TO_sb[:, 1:512])
                nc.vector.tensor_sub(xh[:, 1024:1025], E_sb[:, 0:1], TO_sb[:, 0:1])
                nc.vector.tensor_add(
                    xh[:, 1025:1536], E_sb[:, 513:1024], TO_sb[:, 513:1024]
                )
                nc.scalar.copy(xh[:, 1536:1537], TO_sb[:, 512:513])
                nc.vector.tensor_sub(
                    xh[:, 1537:2048], TO_sb[:, 513:1024], E_sb[:, 513:1024]
                )

            if it < 4:
                nc.sync.dma_start(gw_q[it][:], GW[it])

            # ---------- stage 3: fused GW matmul (t - LAG_GW) ----------
            v = it - LAG_GW
            if v >= 0:
                qv = qv_hist.pop(v)
                osb = opool.tile([P, D], F32, tag="osb")
                for e in range(4):
                    psg = (psumGa if e % 2 == 0 else psumGb).tile(
                        [P, 512], F32, tag="psG"
                    )
                    for pf in range(NPF):
                        nc.tensor.matmul(
                            psg[:],
                            qv[:, pf, :],
                            gw_q[e][:, pf, :],
                            start=(pf == 0),
                            stop=(pf == NPF - 1),
                        )
                    if e % 2 == 0:
                        nc.scalar.copy(osb[:, e * 512 : (e + 1) * 512], psg[:])
                    else:
                        nc.vector.tensor_copy(osb[:, e * 512 : (e + 1) * 512], psg[:])
                nc.scalar.dma_start(out[v * P : (v + 1) * P, :], osb[:])


def _chunked(m):
    r, c = m.shape
    return np.ascontiguousarray(m.reshape(r // P, P, c).transpose(1, 0, 2))


_p = np.arange(1024)
_FM = np.where(
    _p <= 256,
    _p,
    np.where(_p <= 511, 768 - _p, np.where(_p == 512, 512,
             np.where(_p <= 768, 1536 - _p, _p - 256))),
)


def _pack_F(re, im):
    v = np.empty(2048)
    v[0:1024] = re[_FM]
    v[1024] = re[1024]
    v[1025:2048] = im[_FM[1:1024]]
    return v


def _consts():
    if "consts" in _CACHE:
        return _CACHE["consts"]

    k = np.arange(512)

    def cs_cols(dd_base, re_hi, im_lo, im_hi):
        dd = 4 * k + dd_base
        m = np.empty((512, 512))
        m[:, 0:re_hi] = np.cos(2 * np.pi * np.outer(dd, np.arange(re_hi)) / D)
        m[:, re_hi:512] = -np.sin(
            2 * np.pi * np.outer(dd, np.arange(im_lo, im_hi + 1)) / D
        )
        return m

    CB0 = cs_cols(0, 257, 1, 255)
    CB1 = cs_cols(1, 256, 1, 256)
    CB2 = cs_cols(2, 256, 1, 256)
    CB3 = cs_cols(3, 256, 1, 256)
    CBfull = np.concatenate([CB0, CB1, CB2, CB3], axis=0)  # [2048, 512]

    U = np.triu(np.ones((P, P)))
    UI = np.concatenate([U, np.eye(P)], axis=1)
    consts = {
        "CB": _chunked(CBfull.astype(np.float32)).astype(bf16),
        "UI": UI.astype(bf16),
    }
    _CACHE["consts"] = consts
    return consts


def _gw_matrix(w_out):
    f_of_row = np.empty(2048, dtype=np.int64)
    f_of_row[0:1024] = _FM
    f_of_row[1024] = 1024
    f_of_row[1025:2048] = _FM[1:1024]
    alpha = np.where((f_of_row == 0) | (f_of_row == 1024), 1.0, 2.0)
    ang = 2 * np.pi / D * np.outer(f_of_row, np.arange(D))
    G_F = np.empty((2048, D), np.float64)
    G_F[0:1025] = alpha[0:1025, None] * np.cos(ang[0:1025]) / D
    G_F[1025:] = -2.0 * np.sin(ang[1025:]) / D
    GWm = (G_F.astype(np.float32) @ w_out.T.astype(np.float32)).astype(np.float32)
    GWc = _chunked(GWm)  # [P, NPF, D]
    GW4 = np.stack([GWc[:, :, q * 512 : (q + 1) * 512] for q in range(4)])
    return np.ascontiguousarray(GW4).astype(bf16)


def kernel(x, queries, keyvalues, w_out):
    x = np.asarray(x, dtype=np.float32)
    queries = np.asarray(queries, dtype=np.float32)
    keyvalues = np.asarray(keyvalues, dtype=np.float32)
    w_out = np.asarray(w_out, dtype=np.float32)

    if "nc" not in _CACHE:
        _CACHE["nc"] = _build_nc()
    nc = _CACHE["nc"]
    consts = _consts()

    c = (queries * keyvalues).reshape(-1)
    cq = _chunked(_pack_F(c, c).astype(np.float32)[:, None])[:, :, 0]
    GWc = _gw_matrix(w_out)

    in_maps = []
    shards = []
    for b in range(NB):
        for h in range(2):
            shards.append((b, h))
            xs = x[b, h * T : (h + 1) * T]
            xsT = np.ascontiguousarray(xs.T)
            subs = [
                _chunked(np.ascontiguousarray(xsT[j::4])) for j in range(4)
            ]  # each [P, 4, T]
            xfull = np.concatenate(subs, axis=1)  # [P, 16, T]
            xTc = np.ascontiguousarray(
                xfull.reshape(P, NPF, NT, P).transpose(2, 0, 1, 3)
            ).astype(bf16)
            if h == 0:
                c0 = np.zeros((P, NPF), np.float32)
            else:
                F = np.fft.rfft(x[b, :T].sum(axis=0).astype(np.float64))
                c0 = _chunked(
                    _pack_F(F.real, F.imag).astype(np.float32)[:, None]
                )[:, :, 0]
            in_maps.append(
                {
                    "xT": xTc,
                    "CB": consts["CB"],
                    "GW": GWc,
                    "UI": consts["UI"],
                    "CQ": np.ascontiguousarray(cq),
                    "C0": np.ascontiguousarray(c0),
                }
            )

    global _LAST_IN_MAPS
    _LAST_IN_MAPS = in_maps
    res = run_bass_kernel_spmd(nc, in_maps, core_ids=list(range(8)))
    y = np.empty((NB, NS, D), np.float32)
    for i, (b, h) in enumerate(shards):
        y[b, h * T : (h + 1) * T] = res.results[i]["out"]
    return y
